# revision 61
# baseline (speedup 1.0000x reference)
"""Trainium2 Bass kernel for BinaryMemoryTree logits.

logits[b,k,c] = sum_{d,e} q[b,k,d] * memory[b,c,d,e] * v[b,k,e]

Sharding: data-parallel over batch B=8 -> one batch element per NeuronCore.

Strategy (memory-bound problem; rel-err gate 2e-2 leaves precision headroom):
  - host converts q/v/memory to fp16 (10-bit mantissa; |logit| <= ~1e3 is far
    from fp16 range) -> HBM traffic halves vs fp32 (~53us DMA floor/core)
  - host pre-TRANSPOSES q into [g, d, t, p] so the matmul's stationary
    operand streams straight from DMA: no PE transposes, no qT PSUM
    round-trip, no ScalarE qT evacuation
  - per 4-tile chunk: 4 fp16 matmuls (lhsT = qT tile, rhs = [M_0|M_1]) ->
    ctx [k, (c,e)] fp32 in PSUM
  - ScalarE evacuates ctx -> fp16 block buffer; DVE multiplies by v
    (broadcast over c) in 2x mode, one op per MULT_CHUNKS chunks (fewer
    ops: real HW charges a fixed per-op tax the cost model omits)
  - reduce over e per block: binary tree of 2x-mode fp16 adds down to
    TREE_STOP wide, then one 1x tensor_reduce (TensorReduce has no fast
    mode).  Tree stages interleave with multiplies of later blocks so
    dependent DVE ops are never back-to-back.
  - GpSimd (Pool) is deliberately UNUSED: it shares SBUF read/write ports
    with DVE on TRN2, so concurrent Pool work slows DVE by more than Pool
    contributes (measured: +85us on HW vs a cost model that ignores this)
  - logits accumulate fp16 in SBUF; per-block output stores lagged 2 blocks
    so the in-order SP DMA queue never head-of-line blocks the load stream;
    host upcasts the fp16 output to fp32
"""

import sys

sys.path.insert(0, "/opt/trn_rl_repo")

import numpy as np
from concourse import bacc, bass, bass_utils, mybir, tile

B = 8
L = 32768
D = 128
C = 2
P = 128

F32 = mybir.dt.float32
F16 = mybir.dt.float16
BF16 = mybir.dt.bfloat16

import os as _os

IN_DTYPE = _os.environ.get("BMT_IN_DTYPE", "f16")  # f16 | bf16
CTX_BUFS = int(_os.environ.get("BMT_CTX_BUFS", "4"))
IO_BUFS = int(_os.environ.get("BMT_IO_BUFS", "3"))
EVAC_BUFS = int(_os.environ.get("BMT_EVAC_BUFS", "3"))
STORE_LAG = int(_os.environ.get("BMT_STORE_LAG", "3"))  # blocks of store delay
# per-chunk multiply engine, one letter per chunk slot (cycled globally):
#   d = DVE from PSUM, s = ScalarE-evac + DVE 2x, p = ScalarE-evac + Pool
CHUNK_MODES = _os.environ.get("BMT_CHUNK_MODES", "ssss")
# tree-reduce stops at this width and finishes with one 1x tensor_reduce
TREE_STOP = int(_os.environ.get("BMT_TREE_STOP", "16"))
# chunks covered by one DVE multiply op ('s' mode; bigger = fewer op taxes)
MULT_CHUNKS = int(_os.environ.get("BMT_MULT_CHUNKS", "2"))
# blocks sharing one reduce tree (wider tree ops amortize per-op taxes)
TREE_SPAN = int(_os.environ.get("BMT_TREE_SPAN", "2"))

TILES = L // P          # 256 tiles of 128 queries
CHUNK_T = 4             # tiles per PSUM chunk (512 queries)
BLK_T = 16              # tiles per compute block (2048 queries)
NBLK = TILES // BLK_T   # 16 compute blocks
NCH = BLK_T // CHUNK_T  # 4 chunks per block


def _kernel_body(tc, nc, qtd, vd, md, od, stage="full"):
    in_dt = {"f16": F16, "bf16": BF16}[IN_DTYPE]
    ctxmgrs = []

    def pool(*args, **kw):
        p = tc.tile_pool(*args, **kw)
        ctxmgrs.append(p)
        return p.__enter__()

    constp = pool(name="const", bufs=1)
    iop = pool(name="io", bufs=IO_BUFS)
    ctxps = pool(name="ctx_ps", bufs=CTX_BUFS, space="PSUM")
    workp = pool(name="work", bufs=3)
    evacp = pool(name="evac", bufs=EVAC_BUFS)

    # M_cat [d, (c, e)] loaded directly in compute dtype
    m_sb = constp.tile([P, C, D], in_dt)
    nc.sync.dma_start(m_sb[:], md.ap())

    o_view = od.ap().rearrange("(p j) c -> p j c", p=P)
    o_all = constp.tile([P, NBLK, BLK_T, C], in_dt)

    # first group loads ahead of everything else
    qT_g = iop.tile([P, BLK_T, P], in_dt, tag="q")
    v_g = iop.tile([P, BLK_T, D], in_dt, tag="v")
    nc.sync.dma_start(qT_g[:], qtd.ap()[0])
    nc.sync.dma_start(v_g[:], vd.ap()[0])

    # block-reduce jobs lagged one block so Pool/Act multiplies have slack
    pending_reduce = []
    # chunk consumption deferred one chunk to decouple engine streams
    pending_consume = []

    def advance_tree():
        # advance the oldest in-flight block reduce by ONE tree stage; the
        # caller interleaves independent work between calls so dependent
        # stages are never back-to-back on DVE (real HW pays a RAW drain)
        if not pending_reduce:
            return
        src, n, p_blk, o_sb = pending_reduce.pop(0)
        if stage == "mult":
            # light p_blk touch so multiplies aren't dead
            nc.vector.tensor_reduce(
                out=o_sb, in_=p_blk[:, :, :, 0:8],
                axis=mybir.AxisListType.X, op=mybir.AluOpType.max,
            )
            return
        if n > TREE_STOP:
            # one 2x-mode fp16 halving add (TensorReduce has no fast mode)
            n //= 2
            h = evacp.tile([P, TREE_SPAN * BLK_T, C, n], in_dt,
                           tag=f"tree{n}")
            nc.vector.tensor_tensor(
                out=h[:],
                in0=src[:, :, :, 0:n],
                in1=src[:, :, :, n:2 * n],
                op=mybir.AluOpType.add,
            )
            pending_reduce.insert(0, (h[:], n, p_blk, o_sb))
            return
        with nc.allow_low_precision(reason="fp16 logits; gate is 2e-2"):
            if n == 2:
                nc.vector.tensor_tensor(
                    out=o_sb,
                    in0=src[:, :, :, 0],
                    in1=src[:, :, :, 1],
                    op=mybir.AluOpType.add,
                )
            else:
                nc.vector.tensor_reduce(
                    out=o_sb, in_=src,
                    axis=mybir.AxisListType.X, op=mybir.AluOpType.add,
                )

    def consume(gch, ch, ctx, v_sb, p_blk, ctx_blk, o_span):
        mode = CHUNK_MODES[gch % len(CHUNK_MODES)]
        cblk = gch // NCH
        off = (cblk % TREE_SPAN) * BLK_T
        sl = slice(ch * CHUNK_T, (ch + 1) * CHUNK_T)
        psl = slice(off + ch * CHUNK_T, off + (ch + 1) * CHUNK_T)
        if mode == "d":
            # DVE multiplies straight from PSUM (1x), per chunk
            v_bc = v_sb[:, sl, :].unsqueeze(2).broadcast_to(
                [P, CHUNK_T, C, D]
            )
            nc.vector.tensor_tensor(
                out=p_blk[:, psl, :, :],
                in0=ctx[:],
                in1=v_bc,
                op=mybir.AluOpType.mult,
            )
        else:
            # ScalarE evacuates into the block ctx buffer; the 2x DVE
            # multiply is emitted once per MULT_CHUNKS chunks (fewer ops:
            # real HW charges a fixed per-op tax the model omits)
            nc.scalar.copy(ctx_blk[:, sl, :, :], ctx[:])
            if (ch + 1) % MULT_CHUNKS == 0:
                msl = slice((ch + 1 - MULT_CHUNKS) * CHUNK_T,
                            (ch + 1) * CHUNK_T)
                pmsl = slice(off + (ch + 1 - MULT_CHUNKS) * CHUNK_T,
                             off + (ch + 1) * CHUNK_T)
                nt = MULT_CHUNKS * CHUNK_T
                v_bc = v_sb[:, msl, :].unsqueeze(2).broadcast_to(
                    [P, nt, C, D]
                )
                nc.vector.tensor_tensor(
                    out=p_blk[:, pmsl, :, :],
                    in0=ctx_blk[:, msl, :, :],
                    in1=v_bc,
                    op=mybir.AluOpType.mult,
                )
        if ch == NCH - 1 and cblk % TREE_SPAN == TREE_SPAN - 1:
            # span fully multiplied (in emission order) -> reduce is ripe
            pending_reduce.append((p_blk[:], D, p_blk, o_span))

    for blk in range(NBLK):
        if blk > 0:
            qT_g = iop.tile([P, BLK_T, P], in_dt, tag="q")
            v_g = iop.tile([P, BLK_T, D], in_dt, tag="v")
            nc.sync.dma_start(qT_g[:], qtd.ap()[blk])
            nc.sync.dma_start(v_g[:], vd.ap()[blk])

        o_sb = o_all[:, blk]

        if stage == "dma":
            # light touch so loads have a consumer (reduce an 8-wide slice)
            nc.vector.tensor_reduce(
                out=o_sb[:, :, 0], in_=qT_g[:, :, 0:8],
                axis=mybir.AxisListType.X, op=mybir.AluOpType.max,
            )
            nc.vector.tensor_reduce(
                out=o_sb[:, :, 1], in_=v_g[:, :, 0:8],
                axis=mybir.AxisListType.X, op=mybir.AluOpType.max,
            )
            continue

        if stage in ("full", "mult") and blk % TREE_SPAN == 0:
            p_blk = workp.tile([P, TREE_SPAN * BLK_T, C, D], in_dt,
                               tag="pblk")
            o_span = o_all[:, blk:blk + TREE_SPAN].rearrange(
                "p n t c -> p (n t) c"
            )
        ctx_blk = None
        if stage in ("full", "mult"):
            ctx_blk = evacp.tile([P, BLK_T, C, D], in_dt, tag="ctxblk")

        for ch in range(NCH):
            ctx = ctxps.tile([P, CHUNK_T, C, D], F32, tag="ctx")
            for t in range(CHUNK_T):
                tt = ch * CHUNK_T + t
                nc.tensor.matmul(
                    ctx[:, t, :, :],
                    qT_g[:, tt, :],
                    m_sb[:],
                    start=True,
                    stop=True,
                )

            if stage == "matmul":
                # light ctx touch (16-wide slice) so MMs aren't dead
                nc.vector.tensor_reduce(
                    out=o_sb[:, ch * CHUNK_T:(ch + 1) * CHUNK_T, :],
                    in_=ctx[:, :, :, 0:16],
                    axis=mybir.AxisListType.X,
                    op=mybir.AluOpType.max,
                )
                continue

            # tree stages sandwich the independent multiply so dependent
            # DVE ops are never adjacent
            advance_tree()
            pending_consume.append((blk * NCH + ch, ch, ctx, v_g, p_blk, ctx_blk, o_span))
            if len(pending_consume) > 1:
                consume(*pending_consume.pop(0))
            advance_tree()

        # lagged per-block store; by the time the SP queue reaches this
        # DMACopy the data is long since final -> no head-of-line blocking
        sb = blk - STORE_LAG
        if sb >= 0:
            nc.sync.dma_start(o_view[:, sb * BLK_T:(sb + 1) * BLK_T],
                              o_all[:, sb])

    for job in pending_consume:
        consume(*job)
    pending_consume.clear()
    while pending_reduce:
        advance_tree()

    for sb in range(max(NBLK - STORE_LAG, 0), NBLK):
        nc.sync.dma_start(o_view[:, sb * BLK_T:(sb + 1) * BLK_T], o_all[:, sb])

    for p in reversed(ctxmgrs):
        p.__exit__(None, None, None)


_NC_CACHE = {}


def _build(reps=1, stage="full"):
    key = ("nc", reps, stage, IN_DTYPE, CTX_BUFS, CHUNK_MODES, IO_BUFS,
           EVAC_BUFS, STORE_LAG, TREE_STOP, MULT_CHUNKS, TREE_SPAN)
    if key in _NC_CACHE:
        return _NC_CACHE[key]
    in_dt = {"f16": F16, "bf16": BF16}[IN_DTYPE]
    nc = bacc.Bacc("TRN2", target_bir_lowering=False, debug=False)
    # qt: host-pretransposed q, [block, d, tile, k-in-tile]
    qtd = nc.dram_tensor("qt", (NBLK, D, BLK_T, P), in_dt, kind="ExternalInput")
    # v: p-major tiles, [block, k-in-tile, tile, e]
    vd = nc.dram_tensor("v", (NBLK, P, BLK_T, D), in_dt, kind="ExternalInput")
    # m: [d, c, e]
    md = nc.dram_tensor("m", (D, C, D), in_dt, kind="ExternalInput")
    od = nc.dram_tensor("o", (L, C), in_dt, kind="ExternalOutput")
    with tile.TileContext(nc) as tc:
        if reps == 1:
            _kernel_body(tc, nc, qtd, vd, md, od, stage)
        else:
            with tc.For_i(0, reps, 1):
                _kernel_body(tc, nc, qtd, vd, md, od, stage)
    nc.compile()
    _NC_CACHE[key] = nc
    return nc


def _np_dt():
    if IN_DTYPE == "f16":
        return np.float16
    import ml_dtypes

    return ml_dtypes.bfloat16


def _prep_inputs(q, v, memory):
    """Host-side layout + dtype prep for one batch element set."""
    ndt = _np_dt()
    in_maps = []
    for b in range(B):
        # k = p*(L//P) + g*BLK_T + t
        qb = np.asarray(q[b], dtype=np.float32).reshape(P, NBLK, BLK_T, D)
        qt = np.ascontiguousarray(qb.transpose(1, 3, 2, 0)).astype(ndt)
        vb = np.asarray(v[b], dtype=np.float32).reshape(P, NBLK, BLK_T, D)
        vt = np.ascontiguousarray(vb.transpose(1, 0, 2, 3)).astype(ndt)
        mb = np.asarray(memory[b], dtype=np.float32)  # [c, d, e]
        mt = np.ascontiguousarray(mb.transpose(1, 0, 2)).astype(ndt)  # [d,c,e]
        in_maps.append({"qt": qt, "v": vt, "m": mt})
    return in_maps


def kernel(q, v, memory, _trace=False, _reps=1, _stage="full"):
    nc = _build(_reps, _stage)
    in_maps = _prep_inputs(q, v, memory)
    res = bass_utils.run_bass_kernel_spmd(
        nc, in_maps, core_ids=list(range(B)), trace=_trace
    )
    # o rows are k = p*(L//P) + j with j = blk*BLK_T + t == natural k order
    out = np.stack(
        [res.results[b]["o"].astype(np.float32) for b in range(B)]
    )
    if _trace:
        kernel.last_result = res
    return out
